# revision 35
# baseline (speedup 1.0000x reference)
"""Trainium2 Bass kernel for nn_KabschDecoder: per-box sigmoid point weights.

Computes w[b,s,n] = sig(7*(hx-|x'|)) * sig(7*(hy-|y'|)) * sig(7*(hz-|z'|))
where (x',y',z') is lidar point n expressed in box (b,s)'s frame (SE(3),
rotation about z only), and h* are box half-dims.

Strategy (8 NeuronCores, SPMD, no collectives):
  - Shard the N (points) axis 8 ways: each core handles all 256 boxes for
    its 8192-point slice. Host gathers along N (and upcasts f16 -> f32).
  - Host precomputes, per box, the 3 rows of inv(s_T_box) (tiny) and
    7*dims/2; these feed the TensorEngine as weights / ACT as sigmoid bias.
  - Points are packed across all 128 SBUF partitions in PE-quadrant
    layout: 512-point chunk u = 4j+q lives on partitions 32q..32q+7
    (k-rows of the homogeneous coordinate), columns 512j..512j+512, so
    input DMAs run at full width (the DMA cost model is per-partition
    bytes). Matmuls use tile_position rows 0/32/64/96 with per-quadrant
    replicated weights, K=8, M=128 (block-diagonal 2 batches x 64 boxes).
  - Per (group, 2048-col chunk) x 3 components: PE f32r matmuls produce
    x'_c in PSUM; |x'_c| is drained to SBUF by DVE tensor_reduce
    (apply_absolute_value) -- only DVE/ACT may touch PSUM on HW -- with
    the c2 drain moved to an ACT Abs on two iterations to balance engines
    (Abs and Sigmoid share the ACT table, so no reload); ACT evaluates
    sig(-7*|x'| + 7h) via per-partition bias; Pool multiplies the three
    f16 factors; SP writes f16 rows to HBM. The final iteration's product
    + store is split 4 ways across engines to overlap DMA init latencies.
    Steady state is DVE/ACT-balanced at ~49.5us of engine busy per core.
"""
import sys

sys.path.insert(0, "/opt/trn_rl_repo")

import numpy as np

import concourse.bass as bass
import concourse.tile as tile
from concourse import mybir
from concourse.bass_utils import run_bass_kernel_spmd

B, S, N = 4, 64, 65536
NCORES = 8
NSH = N // NCORES          # 8192 points per core
FD = 2048                  # free-dim chunk (4 PSUM banks)
NPAIR = B // 2             # batches packed per 128-row group
CHUNK = 512                # points per PE matmul / quadrant chunk
NQ = 4                     # PE quadrants (tile_position rows 0/32/64/96)
SIGMOID_SLOPE = 7.0
ACT_DRAIN_ITERS = (3, 6)   # iterations whose c2 PSUM-drain runs on ACT (engine balance)
HALF = 0.5                 # OBJ_DIM_SCALE * 0.5

F32 = mybir.dt.float32
F32R = mybir.dt.float32r
F16 = mybir.dt.float16


MAX_WAITS_PER_INST = 1


def _split_sync_waits(nc: bass.Bass, limit: int = MAX_WAITS_PER_INST):
    """This walrus build rejects instructions carrying more than ~1 sync
    wait command. Move excess waits onto same-engine NOPs inserted just
    before the over-subscribed instruction (engines execute their queue in
    order, so this is semantically identical)."""
    uid = 0
    for fn in nc.m.functions:
        for blk in fn.blocks:
            insts = list(blk.instructions)
            out = []
            changed = False
            for ins in insts:
                si = ins.sync_info
                if si is not None and si.on_wait and len(si.on_wait) > limit:
                    waits = list(si.on_wait)
                    keep = waits[:limit]
                    rest = waits[limit:]
                    ins.sync_info = mybir.SyncInfo(
                        on_wait=keep, on_update=list(si.on_update)
                    )
                    for i in range(0, len(rest), limit):
                        nop = mybir.InstNoOp(
                            name=f"waitsplit-{uid}",
                            ins=[],
                            outs=[],
                            engine=ins.engine,
                        )
                        nop.sync_info = mybir.SyncInfo(
                            on_wait=list(rest[i : i + limit]), on_update=[]
                        )
                        uid += 1
                        out.append(nop)
                    changed = True
                out.append(ins)
            if changed:
                blk.instructions = out


def _build_nc() -> bass.Bass:
    nc = bass.Bass("TRN2", target_bir_lowering=False, debug=False)
    rhs_d = nc.dram_tensor("rhs", [NPAIR, 128, FD], F32R, kind="ExternalInput").ap()
    wpack_d = nc.dram_tensor(
        "wpack", [128, NPAIR * 3 * 128], F32R, kind="ExternalInput"
    ).ap()
    hpack_d = nc.dram_tensor("hpack", [128, NPAIR * 6], F32, kind="ExternalInput").ap()
    out_d = nc.dram_tensor("out", [2 * S * NPAIR, NSH], F16, kind="ExternalOutput").ap()

    with tile.TileContext(nc) as tc:
        with (
            tc.tile_pool(name="const", bufs=1) as cpool,
            tc.tile_pool(name="psum", bufs=2, space="PSUM") as ppool,
            tc.tile_pool(name="sig", bufs=3) as spool,
            tc.tile_pool(name="fin", bufs=3) as fpool,
        ):
            # --- inputs, spread across engine DMA queues for parallel load
            wp = cpool.tile([128, NPAIR * 3 * 128], F32R, tag="wp")
            for g in range(NPAIR):
                lo, hi = g * 3 * 128, (g + 1) * 3 * 128
                nc.scalar.dma_start(wp[:, lo:hi], wpack_d[:, lo:hi])
            hp = cpool.tile([128, NPAIR * 6], F32, tag="hp")
            nc.scalar.dma_start(hp[:], hpack_d)

            rhs_sb = []
            for g in range(NPAIR):
                r = cpool.tile([128, FD], F32R, tag=f"rhs{g}")
                for jc in range(FD // CHUNK):  # 4 col-chunks of 512
                    lo, hi = jc * CHUNK, (jc + 1) * CHUNK
                    eng = nc.sync if (g == 1 or jc < 2) else nc.gpsimd
                    eng.dma_start(r[:, lo:hi], rhs_d[g][:, lo:hi])
                rhs_sb.append(r)

            # warm the sigmoid ACT table before the first real activation
            warm = spool.tile([128, 1], F16, tag="warm")
            nc.scalar.activation(
                warm[:],
                hp[:, 0:1],
                mybir.ActivationFunctionType.Sigmoid,
                bias=hp[:, 0:1],
                scale=-SIGMOID_SLOPE,
            )

            nj = NSH // FD

            def emit_front(g, j, act_drain=False):
                """Matmuls + (|x'|-h) + sigmoid(s) for iteration (g, j).

                The abs stage computes |x'_c| - h_c via tensor_scalar
                (op0=abs_max vs 0, op1=subtract per-partition h), writing into
                a contiguous [128, 3*FD] tile so a single bias-free ACT
                instruction can evaluate sig(-7 * t) for all 3 components.
                Returns the ws3 tile (f16 [128, 3*FD]).
                """
                t3 = spool.tile([128, 3 * FD], F32, tag="t3", name=f"t3_{g}_{j}")
                ws3 = spool.tile([128, 3 * FD], F16, tag="ws3", name=f"ws3_{g}_{j}")
                for c in range(3):
                    v = ppool.tile([128, FD], F32, tag="v", name=f"v_{g}_{j}_{c}")
                    woff = (g * 3 + c) * 128
                    for q in range(NQ):
                        # chunk u = 4j+q lives on partitions 32q..32q+8
                        nc.tensor.matmul(
                            v[:, q * CHUNK : (q + 1) * CHUNK],
                            wp[32 * q : 32 * q + 8, woff : woff + 128],
                            rhs_sb[g][32 * q : 32 * q + 8, j * CHUNK : (j + 1) * CHUNK],
                            start=True,
                            stop=True,
                            tile_position=(32 * q, 0),
                        )
                    # PSUM drain: only DVE and ACT may touch PSUM on HW
                    # (the BIR verifier rejects GPSIMD<->PSUM). t3[:, c] = |v|
                    # via DVE tensor_reduce-abs, or ACT Abs on drain-balance
                    # iterations; the sigmoid applies scale -7 and bias 7h.
                    hcol = g * 3 + c
                    if act_drain and c == 2:
                        nc.scalar.activation(
                            t3[:, 2 * FD : 3 * FD],
                            v[:],
                            mybir.ActivationFunctionType.Abs,
                        )
                    else:
                        nc.vector.tensor_reduce(
                            t3[:, c * FD : (c + 1) * FD],
                            v[:].rearrange("p (f one) -> p f one", one=1),
                            axis=mybir.AxisListType.X,
                            op=mybir.AluOpType.max,
                            apply_absolute_value=True,
                        )
                    nc.scalar.activation(
                        ws3[:, c * FD : (c + 1) * FD],
                        t3[:, c * FD : (c + 1) * FD],
                        mybir.ActivationFunctionType.Sigmoid,
                        bias=hp[:, 6 + hcol : 6 + hcol + 1],
                        scale=-SIGMOID_SLOPE,
                    )
                return ws3

            def emit_back(g, j, ws3, final=False):
                """Products + output DMA for iteration (g, j).

                For the final iteration the second product and its DMA are
                split 4 ways across engines so the DMA init latencies overlap
                in the drain."""
                wxy = spool.tile([128, FD], F16, tag="wxy", name=f"wxy_{g}_{j}")
                nc.gpsimd.tensor_tensor(
                    wxy[:],
                    ws3[:, 0:FD],
                    ws3[:, FD : 2 * FD],
                    op=mybir.AluOpType.mult,
                )
                wfin = fpool.tile([128, FD], F16, tag="wfin", name=f"wfin_{g}_{j}")
                if not final:
                    nc.gpsimd.tensor_tensor(
                        wfin[:],
                        wxy[:],
                        ws3[:, 2 * FD : 3 * FD],
                        op=mybir.AluOpType.mult,
                    )
                    nc.sync.dma_start(
                        out_d[g * 128 : (g + 1) * 128, j * FD : (j + 1) * FD],
                        wfin[:],
                    )
                else:
                    dma_engs = [nc.sync, nc.scalar, nc.sync, nc.scalar]
                    nh = FD // 4
                    for h in range(4):
                        lo, hi = h * nh, (h + 1) * nh
                        nc.gpsimd.tensor_tensor(
                            wfin[:, lo:hi],
                            wxy[:, lo:hi],
                            ws3[:, 2 * FD + lo : 2 * FD + hi],
                            op=mybir.AluOpType.mult,
                        )
                        dma_engs[h].dma_start(
                            out_d[g * 128 : (g + 1) * 128, j * FD + lo : j * FD + hi],
                            wfin[:, lo:hi],
                        )

            # 1-iteration software pipeline: the DVE multiplies for iteration
            # t-1 are emitted after iteration t's abs/sigmoids so the DVE
            # in-order queue never puts a mult in front of the next abs.
            prev = None
            niter = NPAIR * nj
            for it in range(niter):
                g, j = it // nj, it % nj
                # iter 0: 3 separate sigmoids (earliest ACT start);
                # middle iters: one merged sigmoid (fewest bubbles);
                # last iters: [c0c1]+[c2] so wxy overlaps the c2 sigmoid
                ws3 = emit_front(g, j, act_drain=it in ACT_DRAIN_ITERS)
                if prev is not None:
                    emit_back(*prev)
                prev = (g, j, ws3)
            emit_back(*prev, final=True)
    _split_sync_waits(nc)
    return nc


_NC_CACHE = None


def _get_nc():
    global _NC_CACHE
    if _NC_CACHE is None:
        _NC_CACHE = _build_nc()
    return _NC_CACHE


def _host_prep(pos, dims, rot, points, valid_mask):
    pos = np.asarray(pos, dtype=np.float32)
    dims = np.asarray(dims, dtype=np.float32)
    rot = np.asarray(rot, dtype=np.float32)
    points = np.asarray(points, dtype=np.float32)
    valid_mask = np.asarray(valid_mask)

    pts = np.where(valid_mask[..., None], points, np.float32(0.0))  # (B,N,3)

    c = np.cos(rot[..., 0])  # (B,S)
    s = np.sin(rot[..., 0])
    tx, ty, tz = pos[..., 0], pos[..., 1], pos[..., 2]
    zero = np.zeros_like(c)
    one = np.ones_like(c)
    # rows of inv(s_T_box) (top 3 rows)
    rows = np.stack(
        [
            np.stack([c, s, zero, -(c * tx + s * ty)], axis=-1),
            np.stack([-s, c, zero, s * tx - c * ty], axis=-1),
            np.stack([zero, zero, one, -tz], axis=-1),
        ],
        axis=-2,
    )  # (B, S, 3, 4)
    rows = rows.astype(np.float32)

    # Block-diagonal PE weights, replicated per PE quadrant:
    # wpack[32q + k, ((g*3)+c)*128 + m]
    wq = np.zeros((8, NPAIR * 3 * 128), dtype=np.float32)
    for g in range(NPAIR):
        for half in range(2):
            b = 2 * g + half
            blk = rows[b].transpose(1, 2, 0)  # (3, 4, S)
            for comp in range(3):
                off = (g * 3 + comp) * 128 + 64 * half
                wq[4 * half : 4 * half + 4, off : off + S] = blk[comp]
    wpack = np.zeros((128, NPAIR * 3 * 128), dtype=np.float32)
    for q in range(NQ):
        wpack[32 * q : 32 * q + 8] = wq

    # Per-partition box half-dims: hpack[m, g*3+c] = dims/2 (used by the
    # DVE |x'|-h tensor_scalar path); hpack[m, 6 + g*3+c] = 7*dims/2 (used
    # as sigmoid bias by the ACT Abs-drain path)
    hpack = np.zeros((128, NPAIR * 6), dtype=np.float32)
    harr = (HALF * dims).astype(np.float32)  # (B,S,3)
    for g in range(NPAIR):
        for half in range(2):
            b = 2 * g + half
            hpack[64 * half : 64 * half + S, g * 3 : g * 3 + 3] = harr[b]
            hpack[64 * half : 64 * half + S, 6 + g * 3 : 6 + g * 3 + 3] = (
                SIGMOID_SLOPE * harr[b]
            )

    # rhs[g, k, n]: homogeneous points of the two batches stacked along K
    rhs = np.zeros((NPAIR, 8, N), dtype=np.float32)
    for g in range(NPAIR):
        for half in range(2):
            b = 2 * g + half
            rhs[g, 4 * half : 4 * half + 3] = pts[b].T
            rhs[g, 4 * half + 3] = 1.0
    return rhs, wpack, hpack


def _pack_rhs_core(rhs_core):
    """[NPAIR, 8, NSH] -> quadrant layout [NPAIR, 128, FD]:
    chunk u = 4j+q (512 points) -> partitions 32q..32q+8, cols 512j..512j+512.
    """
    out = np.zeros((NPAIR, 128, FD), dtype=np.float32)
    for g in range(NPAIR):
        # (8, NSH) -> (8, nj, NQ, CHUNK)
        r = rhs_core[g].reshape(8, NSH // FD, NQ, CHUNK)
        for q in range(NQ):
            # cols j*CHUNK..: chunk 4j+q
            out[g, 32 * q : 32 * q + 8] = r[:, :, q, :].reshape(8, -1)
    return out


def make_in_maps(pos, dims, rot, points, valid_mask):
    rhs, wpack, hpack = _host_prep(pos, dims, rot, points, valid_mask)
    in_maps = []
    for core in range(NCORES):
        n0 = core * NSH
        in_maps.append(
            {
                "rhs": _pack_rhs_core(rhs[:, :, n0 : n0 + NSH]),
                "wpack": wpack,
                "hpack": hpack,
            }
        )
    return in_maps


def unshard(results):
    out = np.empty((B * S, N), dtype=np.float32)
    for core in range(NCORES):
        n0 = core * NSH
        out[:, n0 : n0 + NSH] = results[core]["out"].astype(np.float32)
    return out.reshape(B, S, N)


def core0_slice():
    return np.s_[:, :, 0:NSH]


def kernel(pos, dims, rot, points, valid_mask, _want_trace=False):
    in_maps = make_in_maps(pos, dims, rot, points, valid_mask)
    nc = _get_nc()
    res = run_bass_kernel_spmd(
        nc, in_maps, core_ids=list(range(NCORES)), trace=_want_trace
    )
    result = unshard(res.results)
    if _want_trace:
        return result, res
    return result


# revision 42
# speedup vs baseline: 1.0223x; 1.0223x over previous
"""Trainium2 Bass kernel for nn_KabschDecoder: per-box sigmoid point weights.

Computes w[b,s,n] = sig(7*(hx-|x'|)) * sig(7*(hy-|y'|)) * sig(7*(hz-|z'|))
where (x',y',z') is lidar point n expressed in box (b,s)'s frame (SE(3),
rotation about z only), and h* are box half-dims.

Strategy (8 NeuronCores, SPMD, no collectives):
  - Shard the N (points) axis 8 ways: each core handles all 256 boxes for
    its 8192-point slice. Host gathers along N (and upcasts f16 -> f32).
  - Host precomputes, per box, the 3 rows of inv(s_T_box) (tiny) and
    7*dims/2; these feed the TensorEngine as weights / ACT as sigmoid bias.
  - Points are packed across all 128 SBUF partitions in PE-quadrant
    layout: 512-point chunk u = 4j+q lives on partitions 32q..32q+7
    (k-rows of the homogeneous coordinate), columns 512j..512j+512, so
    input DMAs run at full width (the DMA cost model is per-partition
    bytes). Matmuls use tile_position rows 0/32/64/96 with per-quadrant
    replicated weights, K=8, M=128 (block-diagonal 2 batches x 64 boxes).
  - Per (group, 2048-col chunk) x 3 components: PE f32r matmuls produce
    x'_c in PSUM; |x'_c| is drained to SBUF by DVE tensor_reduce
    (apply_absolute_value) -- only DVE/ACT may touch PSUM on HW -- with
    the c2 drain moved to an ACT Abs on two iterations to balance engines
    (Abs and Sigmoid share the ACT table, so no reload); ACT evaluates
    sig(-7*|x'| + 7h) via per-partition bias; Pool multiplies the three
    f16 factors; SP writes f16 rows to HBM. The final iteration's product
    + store is split 4 ways across engines to overlap DMA init latencies.
    Steady state is DVE/ACT-balanced at ~49.5us of engine busy per core.
"""
import sys

sys.path.insert(0, "/opt/trn_rl_repo")

import numpy as np

import concourse.bass as bass
import concourse.tile as tile
from concourse import mybir
from concourse.bass_utils import run_bass_kernel_spmd

B, S, N = 4, 64, 65536
NCORES = 8
NSH = N // NCORES          # 8192 points per core
FD = 2048                  # free-dim chunk (4 PSUM banks)
NPAIR = B // 2             # batches packed per 128-row group
CHUNK = 512                # points per PE matmul / quadrant chunk
NQ = 4                     # PE quadrants (tile_position rows 0/32/64/96)
SIGMOID_SLOPE = 7.0
ACT_DRAIN_ITERS = (3, 6)   # iterations whose c2 PSUM-drain runs on ACT (engine balance)
HALF = 0.5                 # OBJ_DIM_SCALE * 0.5

F32 = mybir.dt.float32
F32R = mybir.dt.float32r
F16 = mybir.dt.float16


MAX_WAITS_PER_INST = 1


def _split_sync_waits(nc: bass.Bass, limit: int = MAX_WAITS_PER_INST):
    """This walrus build rejects instructions carrying more than ~1 sync
    wait command. Move excess waits onto same-engine NOPs inserted just
    before the over-subscribed instruction (engines execute their queue in
    order, so this is semantically identical)."""
    uid = 0
    for fn in nc.m.functions:
        for blk in fn.blocks:
            insts = list(blk.instructions)
            out = []
            changed = False
            for ins in insts:
                si = ins.sync_info
                if si is not None and si.on_wait and len(si.on_wait) > limit:
                    waits = list(si.on_wait)
                    keep = waits[:limit]
                    rest = waits[limit:]
                    ins.sync_info = mybir.SyncInfo(
                        on_wait=keep, on_update=list(si.on_update)
                    )
                    for i in range(0, len(rest), limit):
                        nop = mybir.InstNoOp(
                            name=f"waitsplit-{uid}",
                            ins=[],
                            outs=[],
                            engine=ins.engine,
                        )
                        nop.sync_info = mybir.SyncInfo(
                            on_wait=list(rest[i : i + limit]), on_update=[]
                        )
                        uid += 1
                        out.append(nop)
                    changed = True
                out.append(ins)
            if changed:
                blk.instructions = out


def _build_nc() -> bass.Bass:
    nc = bass.Bass("TRN2", target_bir_lowering=False, debug=False)
    rhs_d = nc.dram_tensor("rhs", [NPAIR, 128, FD], F32R, kind="ExternalInput").ap()
    wpack_d = nc.dram_tensor(
        "wpack", [128, NPAIR * 3 * 128], F32R, kind="ExternalInput"
    ).ap()
    hpack_d = nc.dram_tensor("hpack", [128, NPAIR * 6], F32, kind="ExternalInput").ap()
    out_d = nc.dram_tensor("out", [2 * S * NPAIR, NSH], F16, kind="ExternalOutput").ap()

    with tile.TileContext(nc) as tc:
        with (
            tc.tile_pool(name="const", bufs=1) as cpool,
            tc.tile_pool(name="psum", bufs=2, space="PSUM") as ppool,
            tc.tile_pool(name="sig", bufs=3) as spool,
            tc.tile_pool(name="fin", bufs=3) as fpool,
        ):
            # --- inputs, spread across engine DMA queues for parallel load
            wp = cpool.tile([128, NPAIR * 3 * 128], F32R, tag="wp")
            for g in range(NPAIR):
                lo, hi = g * 3 * 128, (g + 1) * 3 * 128
                nc.scalar.dma_start(wp[:, lo:hi], wpack_d[:, lo:hi])
            hp = cpool.tile([128, NPAIR * 6], F32, tag="hp")
            nc.scalar.dma_start(hp[:], hpack_d)

            rhs_sb = []
            for g in range(NPAIR):
                r = cpool.tile([128, FD], F32R, tag=f"rhs{g}")
                for jc in range(FD // CHUNK):  # 4 col-chunks of 512
                    lo, hi = jc * CHUNK, (jc + 1) * CHUNK
                    eng = nc.sync if (g == 1 or jc < 2) else nc.gpsimd
                    eng.dma_start(r[:, lo:hi], rhs_d[g][:, lo:hi])
                rhs_sb.append(r)

            # warm the sigmoid ACT table before the first real activation
            warm = spool.tile([128, 1], F16, tag="warm")
            nc.scalar.activation(
                warm[:],
                hp[:, 0:1],
                mybir.ActivationFunctionType.Sigmoid,
                bias=hp[:, 0:1],
                scale=-SIGMOID_SLOPE,
            )

            nj = NSH // FD

            def emit_front(g, j, act_drain=False, final=False):
                """Matmuls + (|x'|-h) + sigmoid(s) for iteration (g, j).

                The abs stage computes |x'_c| - h_c via tensor_scalar
                (op0=abs_max vs 0, op1=subtract per-partition h), writing into
                a contiguous [128, 3*FD] tile so a single bias-free ACT
                instruction can evaluate sig(-7 * t) for all 3 components.
                Returns the ws3 tile (f16 [128, 3*FD]).
                """
                t3 = spool.tile([128, 3 * FD], F32, tag="t3", name=f"t3_{g}_{j}")
                ws3 = spool.tile([128, 3 * FD], F16, tag="ws3", name=f"ws3_{g}_{j}")
                for c in range(3):
                    v = ppool.tile([128, FD], F32, tag="v", name=f"v_{g}_{j}_{c}")
                    woff = (g * 3 + c) * 128
                    for q in range(NQ):
                        # chunk u = 4j+q lives on partitions 32q..32q+8
                        nc.tensor.matmul(
                            v[:, q * CHUNK : (q + 1) * CHUNK],
                            wp[32 * q : 32 * q + 8, woff : woff + 128],
                            rhs_sb[g][32 * q : 32 * q + 8, j * CHUNK : (j + 1) * CHUNK],
                            start=True,
                            stop=True,
                            tile_position=(32 * q, 0),
                        )
                    # PSUM drain: only DVE and ACT may touch PSUM on HW
                    # (the BIR verifier rejects GPSIMD<->PSUM). t3[:, c] = |v|
                    # via DVE tensor_reduce-abs, or ACT Abs on drain-balance
                    # iterations; the sigmoid applies scale -7 and bias 7h.
                    hcol = g * 3 + c
                    if act_drain and c == 2:
                        nc.scalar.activation(
                            t3[:, 2 * FD : 3 * FD],
                            v[:],
                            mybir.ActivationFunctionType.Abs,
                        )
                    else:
                        nc.vector.tensor_reduce(
                            t3[:, c * FD : (c + 1) * FD],
                            v[:].rearrange("p (f one) -> p f one", one=1),
                            axis=mybir.AxisListType.X,
                            op=mybir.AluOpType.max,
                            apply_absolute_value=True,
                        )
                    if final and c == 2:
                        # tail: halve the last sigmoid so the products can
                        # start after the first half
                        half = FD // 2
                        for lo in (0, half):
                            nc.scalar.activation(
                                ws3[:, 2 * FD + lo : 2 * FD + lo + half],
                                t3[:, 2 * FD + lo : 2 * FD + lo + half],
                                mybir.ActivationFunctionType.Sigmoid,
                                bias=hp[:, 6 + hcol : 6 + hcol + 1],
                                scale=-SIGMOID_SLOPE,
                            )
                    else:
                        nc.scalar.activation(
                            ws3[:, c * FD : (c + 1) * FD],
                            t3[:, c * FD : (c + 1) * FD],
                            mybir.ActivationFunctionType.Sigmoid,
                            bias=hp[:, 6 + hcol : 6 + hcol + 1],
                            scale=-SIGMOID_SLOPE,
                        )
                return ws3

            def emit_back(g, j, ws3, final=False):
                """Products + output DMA for iteration (g, j).

                For the final iteration the second product and its DMA are
                split 4 ways across engines so the DMA init latencies overlap
                in the drain."""
                wxy = spool.tile([128, FD], F16, tag="wxy", name=f"wxy_{g}_{j}")
                nc.gpsimd.tensor_tensor(
                    wxy[:],
                    ws3[:, 0:FD],
                    ws3[:, FD : 2 * FD],
                    op=mybir.AluOpType.mult,
                )
                wfin = fpool.tile([128, FD], F16, tag="wfin", name=f"wfin_{g}_{j}")
                if not final:
                    nc.gpsimd.tensor_tensor(
                        wfin[:],
                        wxy[:],
                        ws3[:, 2 * FD : 3 * FD],
                        op=mybir.AluOpType.mult,
                    )
                    nc.sync.dma_start(
                        out_d[g * 128 : (g + 1) * 128, j * FD : (j + 1) * FD],
                        wfin[:],
                    )
                else:
                    dma_engs = [nc.sync, nc.scalar, nc.sync, nc.scalar]
                    mul_engs = [nc.gpsimd, nc.vector, nc.gpsimd, nc.vector]
                    nh = FD // 4
                    for h in range(4):
                        lo, hi = h * nh, (h + 1) * nh
                        mul_engs[h].tensor_tensor(
                            wfin[:, lo:hi],
                            wxy[:, lo:hi],
                            ws3[:, 2 * FD + lo : 2 * FD + hi],
                            op=mybir.AluOpType.mult,
                        )
                        dma_engs[h].dma_start(
                            out_d[g * 128 : (g + 1) * 128, j * FD + lo : j * FD + hi],
                            wfin[:, lo:hi],
                        )

            # 1-iteration software pipeline: the DVE multiplies for iteration
            # t-1 are emitted after iteration t's abs/sigmoids so the DVE
            # in-order queue never puts a mult in front of the next abs.
            prev = None
            niter = NPAIR * nj
            for it in range(niter):
                g, j = it // nj, it % nj
                # iter 0: 3 separate sigmoids (earliest ACT start);
                # middle iters: one merged sigmoid (fewest bubbles);
                # last iters: [c0c1]+[c2] so wxy overlaps the c2 sigmoid
                ws3 = emit_front(
                    g, j, act_drain=it in ACT_DRAIN_ITERS, final=it == niter - 1
                )
                if prev is not None:
                    emit_back(*prev)
                prev = (g, j, ws3)
            emit_back(*prev, final=True)
    _split_sync_waits(nc)
    return nc


_NC_CACHE = None


def _get_nc():
    global _NC_CACHE
    if _NC_CACHE is None:
        _NC_CACHE = _build_nc()
    return _NC_CACHE


def _host_prep(pos, dims, rot, points, valid_mask):
    pos = np.asarray(pos, dtype=np.float32)
    dims = np.asarray(dims, dtype=np.float32)
    rot = np.asarray(rot, dtype=np.float32)
    points = np.asarray(points, dtype=np.float32)
    valid_mask = np.asarray(valid_mask)

    pts = np.where(valid_mask[..., None], points, np.float32(0.0))  # (B,N,3)

    c = np.cos(rot[..., 0])  # (B,S)
    s = np.sin(rot[..., 0])
    tx, ty, tz = pos[..., 0], pos[..., 1], pos[..., 2]
    zero = np.zeros_like(c)
    one = np.ones_like(c)
    # rows of inv(s_T_box) (top 3 rows)
    rows = np.stack(
        [
            np.stack([c, s, zero, -(c * tx + s * ty)], axis=-1),
            np.stack([-s, c, zero, s * tx - c * ty], axis=-1),
            np.stack([zero, zero, one, -tz], axis=-1),
        ],
        axis=-2,
    )  # (B, S, 3, 4)
    rows = rows.astype(np.float32)

    # Block-diagonal PE weights, replicated per PE quadrant:
    # wpack[32q + k, ((g*3)+c)*128 + m]
    wq = np.zeros((8, NPAIR * 3 * 128), dtype=np.float32)
    for g in range(NPAIR):
        for half in range(2):
            b = 2 * g + half
            blk = rows[b].transpose(1, 2, 0)  # (3, 4, S)
            for comp in range(3):
                off = (g * 3 + comp) * 128 + 64 * half
                wq[4 * half : 4 * half + 4, off : off + S] = blk[comp]
    wpack = np.zeros((128, NPAIR * 3 * 128), dtype=np.float32)
    for q in range(NQ):
        wpack[32 * q : 32 * q + 8] = wq

    # Per-partition box half-dims: hpack[m, g*3+c] = dims/2 (used by the
    # DVE |x'|-h tensor_scalar path); hpack[m, 6 + g*3+c] = 7*dims/2 (used
    # as sigmoid bias by the ACT Abs-drain path)
    hpack = np.zeros((128, NPAIR * 6), dtype=np.float32)
    harr = (HALF * dims).astype(np.float32)  # (B,S,3)
    for g in range(NPAIR):
        for half in range(2):
            b = 2 * g + half
            hpack[64 * half : 64 * half + S, g * 3 : g * 3 + 3] = harr[b]
            hpack[64 * half : 64 * half + S, 6 + g * 3 : 6 + g * 3 + 3] = (
                SIGMOID_SLOPE * harr[b]
            )

    # rhs[g, k, n]: homogeneous points of the two batches stacked along K
    rhs = np.zeros((NPAIR, 8, N), dtype=np.float32)
    for g in range(NPAIR):
        for half in range(2):
            b = 2 * g + half
            rhs[g, 4 * half : 4 * half + 3] = pts[b].T
            rhs[g, 4 * half + 3] = 1.0
    return rhs, wpack, hpack


def _pack_rhs_core(rhs_core):
    """[NPAIR, 8, NSH] -> quadrant layout [NPAIR, 128, FD]:
    chunk u = 4j+q (512 points) -> partitions 32q..32q+8, cols 512j..512j+512.
    """
    out = np.zeros((NPAIR, 128, FD), dtype=np.float32)
    for g in range(NPAIR):
        # (8, NSH) -> (8, nj, NQ, CHUNK)
        r = rhs_core[g].reshape(8, NSH // FD, NQ, CHUNK)
        for q in range(NQ):
            # cols j*CHUNK..: chunk 4j+q
            out[g, 32 * q : 32 * q + 8] = r[:, :, q, :].reshape(8, -1)
    return out


def make_in_maps(pos, dims, rot, points, valid_mask):
    rhs, wpack, hpack = _host_prep(pos, dims, rot, points, valid_mask)
    in_maps = []
    for core in range(NCORES):
        n0 = core * NSH
        in_maps.append(
            {
                "rhs": _pack_rhs_core(rhs[:, :, n0 : n0 + NSH]),
                "wpack": wpack,
                "hpack": hpack,
            }
        )
    return in_maps


def unshard(results):
    out = np.empty((B * S, N), dtype=np.float32)
    for core in range(NCORES):
        n0 = core * NSH
        out[:, n0 : n0 + NSH] = results[core]["out"].astype(np.float32)
    return out.reshape(B, S, N)


def core0_slice():
    return np.s_[:, :, 0:NSH]


def kernel(pos, dims, rot, points, valid_mask, _want_trace=False):
    in_maps = make_in_maps(pos, dims, rot, points, valid_mask)
    nc = _get_nc()
    res = run_bass_kernel_spmd(
        nc, in_maps, core_ids=list(range(NCORES)), trace=_want_trace
    )
    result = unshard(res.results)
    if _want_trace:
        return result, res
    return result


# revision 44
# speedup vs baseline: 1.0349x; 1.0123x over previous
"""Trainium2 Bass kernel for nn_KabschDecoder: per-box sigmoid point weights.

Computes w[b,s,n] = sig(7*(hx-|x'|)) * sig(7*(hy-|y'|)) * sig(7*(hz-|z'|))
where (x',y',z') is lidar point n expressed in box (b,s)'s frame (SE(3),
rotation about z only), and h* are box half-dims.

Strategy (8 NeuronCores, SPMD, no collectives):
  - Shard the N (points) axis 8 ways: each core handles all 256 boxes for
    its 8192-point slice. Host gathers along N (and upcasts f16 -> f32).
  - Host precomputes, per box, the 3 rows of inv(s_T_box) (tiny) and
    7*dims/2; these feed the TensorEngine as weights / ACT as sigmoid bias.
  - Points are packed across all 128 SBUF partitions in PE-quadrant
    layout: 512-point chunk u = 4j+q lives on partitions 32q..32q+7
    (k-rows of the homogeneous coordinate), columns 512j..512j+512, so
    input DMAs run at full width (the DMA cost model is per-partition
    bytes). Matmuls use tile_position rows 0/32/64/96 with per-quadrant
    replicated weights, K=8, M=128 (block-diagonal 2 batches x 64 boxes).
  - Per (group, 2048-col chunk) x 3 components: PE f32r matmuls produce
    x'_c in PSUM; |x'_c| is drained to SBUF by DVE tensor_reduce
    (apply_absolute_value) -- only DVE/ACT may touch PSUM on HW -- with
    the c2 drain moved to an ACT Abs on two iterations to balance engines
    (Abs and Sigmoid share the ACT table, so no reload); ACT evaluates
    sig(-7*|x'| + 7h) via per-partition bias; Pool multiplies the three
    f16 factors; SP writes f16 rows to HBM. The final iteration's product
    + store is split 4 ways across engines to overlap DMA init latencies.
    Steady state is DVE/ACT-balanced at ~49.5us of engine busy per core.
"""
import sys

sys.path.insert(0, "/opt/trn_rl_repo")

import numpy as np

import concourse.bass as bass
import concourse.tile as tile
from concourse import mybir
from concourse.bass_utils import run_bass_kernel_spmd

B, S, N = 4, 64, 65536
NCORES = 8
NSH = N // NCORES          # 8192 points per core
FD = 2048                  # free-dim chunk (4 PSUM banks)
NPAIR = B // 2             # batches packed per 128-row group
CHUNK = 512                # points per PE matmul / quadrant chunk
NQ = 4                     # PE quadrants (tile_position rows 0/32/64/96)
SIGMOID_SLOPE = 7.0
ACT_DRAIN_ITERS = (3, 6)   # iterations whose c2 PSUM-drain runs on ACT (engine balance)
HALF = 0.5                 # OBJ_DIM_SCALE * 0.5

F32 = mybir.dt.float32
F32R = mybir.dt.float32r
F16 = mybir.dt.float16


MAX_WAITS_PER_INST = 1


def _split_sync_waits(nc: bass.Bass, limit: int = MAX_WAITS_PER_INST):
    """This walrus build rejects instructions carrying more than ~1 sync
    wait command. Move excess waits onto same-engine NOPs inserted just
    before the over-subscribed instruction (engines execute their queue in
    order, so this is semantically identical)."""
    uid = 0
    for fn in nc.m.functions:
        for blk in fn.blocks:
            insts = list(blk.instructions)
            out = []
            changed = False
            for ins in insts:
                si = ins.sync_info
                if si is not None and si.on_wait and len(si.on_wait) > limit:
                    waits = list(si.on_wait)
                    keep = waits[:limit]
                    rest = waits[limit:]
                    ins.sync_info = mybir.SyncInfo(
                        on_wait=keep, on_update=list(si.on_update)
                    )
                    for i in range(0, len(rest), limit):
                        nop = mybir.InstNoOp(
                            name=f"waitsplit-{uid}",
                            ins=[],
                            outs=[],
                            engine=ins.engine,
                        )
                        nop.sync_info = mybir.SyncInfo(
                            on_wait=list(rest[i : i + limit]), on_update=[]
                        )
                        uid += 1
                        out.append(nop)
                    changed = True
                out.append(ins)
            if changed:
                blk.instructions = out


def _build_nc() -> bass.Bass:
    nc = bass.Bass("TRN2", target_bir_lowering=False, debug=False)
    rhs_d = nc.dram_tensor("rhs", [NPAIR, 128, FD], F32R, kind="ExternalInput").ap()
    wpack_d = nc.dram_tensor(
        "wpack", [128, NPAIR * 3 * 128], F32R, kind="ExternalInput"
    ).ap()
    hpack_d = nc.dram_tensor("hpack", [128, NPAIR * 6], F32, kind="ExternalInput").ap()
    out_d = nc.dram_tensor("out", [2 * S * NPAIR, NSH], F16, kind="ExternalOutput").ap()

    with tile.TileContext(nc) as tc:
        with (
            tc.tile_pool(name="const", bufs=1) as cpool,
            tc.tile_pool(name="psum", bufs=2, space="PSUM") as ppool,
            tc.tile_pool(name="sig", bufs=3) as spool,
            tc.tile_pool(name="fin", bufs=3) as fpool,
        ):
            # --- inputs, spread across engine DMA queues for parallel load
            wp = cpool.tile([128, NPAIR * 3 * 128], F32R, tag="wp")
            for g in range(NPAIR):
                lo, hi = g * 3 * 128, (g + 1) * 3 * 128
                nc.scalar.dma_start(wp[:, lo:hi], wpack_d[:, lo:hi])
            hp = cpool.tile([128, NPAIR * 6], F32, tag="hp")
            nc.scalar.dma_start(hp[:], hpack_d)

            rhs_sb = [
                cpool.tile([128, FD], F32R, tag=f"rhs{g}", name=f"rhs_sb{g}")
                for g in range(NPAIR)
            ]

            # Warm the PE p-state ramp with a dummy matmul on zeros at t~0:
            # the ramp clock then hits full speed right as the first real
            # matmuls arrive. The tiles are f32 (an f32r memset fails the
            # walrus ISA check) and bitcast to f32r for the PE. The PSUM
            # output lands in the v ring and is fully overwritten
            # (start=True) before anything reads it.
            zw = cpool.tile([8, CHUNK], F32, tag="zw")
            nc.vector.memset(zw[:], 0.0)
            vwarm = ppool.tile([128, FD], F32, tag="v", name="vwarm")
            nc.tensor.matmul(
                vwarm[:, 0:CHUNK],
                zw[:, 0:128].bitcast(F32R),
                zw[:].bitcast(F32R),
                start=True,
                stop=True,
                tile_position=(0, 0),
            )

            for g in range(NPAIR):
                r = rhs_sb[g]
                for jc in range(FD // CHUNK):  # 4 col-chunks of 512
                    lo, hi = jc * CHUNK, (jc + 1) * CHUNK
                    eng = nc.sync if (g == 1 or jc < 2) else nc.gpsimd
                    eng.dma_start(r[:, lo:hi], rhs_d[g][:, lo:hi])

            # warm the sigmoid ACT table before the first real activation
            warm = spool.tile([128, 1], F16, tag="warm")
            nc.scalar.activation(
                warm[:],
                hp[:, 0:1],
                mybir.ActivationFunctionType.Sigmoid,
                bias=hp[:, 0:1],
                scale=-SIGMOID_SLOPE,
            )

            nj = NSH // FD

            def emit_front(g, j, act_drain=False, final=False):
                """Matmuls + (|x'|-h) + sigmoid(s) for iteration (g, j).

                The abs stage computes |x'_c| - h_c via tensor_scalar
                (op0=abs_max vs 0, op1=subtract per-partition h), writing into
                a contiguous [128, 3*FD] tile so a single bias-free ACT
                instruction can evaluate sig(-7 * t) for all 3 components.
                Returns the ws3 tile (f16 [128, 3*FD]).
                """
                t3 = spool.tile([128, 3 * FD], F32, tag="t3", name=f"t3_{g}_{j}")
                ws3 = spool.tile([128, 3 * FD], F16, tag="ws3", name=f"ws3_{g}_{j}")
                for c in range(3):
                    v = ppool.tile([128, FD], F32, tag="v", name=f"v_{g}_{j}_{c}")
                    woff = (g * 3 + c) * 128
                    for q in range(NQ):
                        # chunk u = 4j+q lives on partitions 32q..32q+8
                        nc.tensor.matmul(
                            v[:, q * CHUNK : (q + 1) * CHUNK],
                            wp[32 * q : 32 * q + 8, woff : woff + 128],
                            rhs_sb[g][32 * q : 32 * q + 8, j * CHUNK : (j + 1) * CHUNK],
                            start=True,
                            stop=True,
                            tile_position=(32 * q, 0),
                        )
                    # PSUM drain: only DVE and ACT may touch PSUM on HW
                    # (the BIR verifier rejects GPSIMD<->PSUM). t3[:, c] = |v|
                    # via DVE tensor_reduce-abs, or ACT Abs on drain-balance
                    # iterations; the sigmoid applies scale -7 and bias 7h.
                    hcol = g * 3 + c
                    if act_drain and c == 2:
                        nc.scalar.activation(
                            t3[:, 2 * FD : 3 * FD],
                            v[:],
                            mybir.ActivationFunctionType.Abs,
                        )
                    else:
                        nc.vector.tensor_reduce(
                            t3[:, c * FD : (c + 1) * FD],
                            v[:].rearrange("p (f one) -> p f one", one=1),
                            axis=mybir.AxisListType.X,
                            op=mybir.AluOpType.max,
                            apply_absolute_value=True,
                        )
                    if final and c == 2:
                        # tail: halve the last sigmoid so the products can
                        # start after the first half
                        half = FD // 2
                        for lo in (0, half):
                            nc.scalar.activation(
                                ws3[:, 2 * FD + lo : 2 * FD + lo + half],
                                t3[:, 2 * FD + lo : 2 * FD + lo + half],
                                mybir.ActivationFunctionType.Sigmoid,
                                bias=hp[:, 6 + hcol : 6 + hcol + 1],
                                scale=-SIGMOID_SLOPE,
                            )
                    else:
                        nc.scalar.activation(
                            ws3[:, c * FD : (c + 1) * FD],
                            t3[:, c * FD : (c + 1) * FD],
                            mybir.ActivationFunctionType.Sigmoid,
                            bias=hp[:, 6 + hcol : 6 + hcol + 1],
                            scale=-SIGMOID_SLOPE,
                        )
                return ws3

            def emit_back(g, j, ws3, final=False):
                """Products + output DMA for iteration (g, j).

                For the final iteration the second product and its DMA are
                split 4 ways across engines so the DMA init latencies overlap
                in the drain."""
                wxy = spool.tile([128, FD], F16, tag="wxy", name=f"wxy_{g}_{j}")
                nc.gpsimd.tensor_tensor(
                    wxy[:],
                    ws3[:, 0:FD],
                    ws3[:, FD : 2 * FD],
                    op=mybir.AluOpType.mult,
                )
                wfin = fpool.tile([128, FD], F16, tag="wfin", name=f"wfin_{g}_{j}")
                if not final:
                    nc.gpsimd.tensor_tensor(
                        wfin[:],
                        wxy[:],
                        ws3[:, 2 * FD : 3 * FD],
                        op=mybir.AluOpType.mult,
                    )
                    nc.sync.dma_start(
                        out_d[g * 128 : (g + 1) * 128, j * FD : (j + 1) * FD],
                        wfin[:],
                    )
                else:
                    dma_engs = [nc.sync, nc.scalar, nc.sync, nc.scalar]
                    mul_engs = [nc.gpsimd, nc.vector, nc.gpsimd, nc.vector]
                    nh = FD // 4
                    for h in range(4):
                        lo, hi = h * nh, (h + 1) * nh
                        mul_engs[h].tensor_tensor(
                            wfin[:, lo:hi],
                            wxy[:, lo:hi],
                            ws3[:, 2 * FD + lo : 2 * FD + hi],
                            op=mybir.AluOpType.mult,
                        )
                        dma_engs[h].dma_start(
                            out_d[g * 128 : (g + 1) * 128, j * FD + lo : j * FD + hi],
                            wfin[:, lo:hi],
                        )

            # 1-iteration software pipeline: the DVE multiplies for iteration
            # t-1 are emitted after iteration t's abs/sigmoids so the DVE
            # in-order queue never puts a mult in front of the next abs.
            prev = None
            niter = NPAIR * nj
            for it in range(niter):
                g, j = it // nj, it % nj
                # iter 0: 3 separate sigmoids (earliest ACT start);
                # middle iters: one merged sigmoid (fewest bubbles);
                # last iters: [c0c1]+[c2] so wxy overlaps the c2 sigmoid
                ws3 = emit_front(
                    g, j, act_drain=it in ACT_DRAIN_ITERS, final=it == niter - 1
                )
                if prev is not None:
                    emit_back(*prev)
                prev = (g, j, ws3)
            emit_back(*prev, final=True)
    _split_sync_waits(nc)
    return nc


_NC_CACHE = None


def _get_nc():
    global _NC_CACHE
    if _NC_CACHE is None:
        _NC_CACHE = _build_nc()
    return _NC_CACHE


def _host_prep(pos, dims, rot, points, valid_mask):
    pos = np.asarray(pos, dtype=np.float32)
    dims = np.asarray(dims, dtype=np.float32)
    rot = np.asarray(rot, dtype=np.float32)
    points = np.asarray(points, dtype=np.float32)
    valid_mask = np.asarray(valid_mask)

    pts = np.where(valid_mask[..., None], points, np.float32(0.0))  # (B,N,3)

    c = np.cos(rot[..., 0])  # (B,S)
    s = np.sin(rot[..., 0])
    tx, ty, tz = pos[..., 0], pos[..., 1], pos[..., 2]
    zero = np.zeros_like(c)
    one = np.ones_like(c)
    # rows of inv(s_T_box) (top 3 rows)
    rows = np.stack(
        [
            np.stack([c, s, zero, -(c * tx + s * ty)], axis=-1),
            np.stack([-s, c, zero, s * tx - c * ty], axis=-1),
            np.stack([zero, zero, one, -tz], axis=-1),
        ],
        axis=-2,
    )  # (B, S, 3, 4)
    rows = rows.astype(np.float32)

    # Block-diagonal PE weights, replicated per PE quadrant:
    # wpack[32q + k, ((g*3)+c)*128 + m]
    wq = np.zeros((8, NPAIR * 3 * 128), dtype=np.float32)
    for g in range(NPAIR):
        for half in range(2):
            b = 2 * g + half
            blk = rows[b].transpose(1, 2, 0)  # (3, 4, S)
            for comp in range(3):
                off = (g * 3 + comp) * 128 + 64 * half
                wq[4 * half : 4 * half + 4, off : off + S] = blk[comp]
    wpack = np.zeros((128, NPAIR * 3 * 128), dtype=np.float32)
    for q in range(NQ):
        wpack[32 * q : 32 * q + 8] = wq

    # Per-partition box half-dims: hpack[m, g*3+c] = dims/2 (used by the
    # DVE |x'|-h tensor_scalar path); hpack[m, 6 + g*3+c] = 7*dims/2 (used
    # as sigmoid bias by the ACT Abs-drain path)
    hpack = np.zeros((128, NPAIR * 6), dtype=np.float32)
    harr = (HALF * dims).astype(np.float32)  # (B,S,3)
    for g in range(NPAIR):
        for half in range(2):
            b = 2 * g + half
            hpack[64 * half : 64 * half + S, g * 3 : g * 3 + 3] = harr[b]
            hpack[64 * half : 64 * half + S, 6 + g * 3 : 6 + g * 3 + 3] = (
                SIGMOID_SLOPE * harr[b]
            )

    # rhs[g, k, n]: homogeneous points of the two batches stacked along K
    rhs = np.zeros((NPAIR, 8, N), dtype=np.float32)
    for g in range(NPAIR):
        for half in range(2):
            b = 2 * g + half
            rhs[g, 4 * half : 4 * half + 3] = pts[b].T
            rhs[g, 4 * half + 3] = 1.0
    return rhs, wpack, hpack


def _pack_rhs_core(rhs_core):
    """[NPAIR, 8, NSH] -> quadrant layout [NPAIR, 128, FD]:
    chunk u = 4j+q (512 points) -> partitions 32q..32q+8, cols 512j..512j+512.
    """
    out = np.zeros((NPAIR, 128, FD), dtype=np.float32)
    for g in range(NPAIR):
        # (8, NSH) -> (8, nj, NQ, CHUNK)
        r = rhs_core[g].reshape(8, NSH // FD, NQ, CHUNK)
        for q in range(NQ):
            # cols j*CHUNK..: chunk 4j+q
            out[g, 32 * q : 32 * q + 8] = r[:, :, q, :].reshape(8, -1)
    return out


def make_in_maps(pos, dims, rot, points, valid_mask):
    rhs, wpack, hpack = _host_prep(pos, dims, rot, points, valid_mask)
    in_maps = []
    for core in range(NCORES):
        n0 = core * NSH
        in_maps.append(
            {
                "rhs": _pack_rhs_core(rhs[:, :, n0 : n0 + NSH]),
                "wpack": wpack,
                "hpack": hpack,
            }
        )
    return in_maps


def unshard(results):
    out = np.empty((B * S, N), dtype=np.float32)
    for core in range(NCORES):
        n0 = core * NSH
        out[:, n0 : n0 + NSH] = results[core]["out"].astype(np.float32)
    return out.reshape(B, S, N)


def core0_slice():
    return np.s_[:, :, 0:NSH]


def kernel(pos, dims, rot, points, valid_mask, _want_trace=False):
    in_maps = make_in_maps(pos, dims, rot, points, valid_mask)
    nc = _get_nc()
    res = run_bass_kernel_spmd(
        nc, in_maps, core_ids=list(range(NCORES)), trace=_want_trace
    )
    result = unshard(res.results)
    if _want_trace:
        return result, res
    return result
